# revision 10
# baseline (speedup 1.0000x reference)
"""AttnVLAD layer on 8 Trainium2 NeuronCores.

Data-parallel over batch: b=32 samples -> 4 per core. Host precomputes
fp16 copies of x in both layouts (d-major for mm1, n-major for mm2) plus
the fp16 split of q = alpha * centers/||centers||, so the device does no
casting or transposing of x. The global L2 normalize is folded into the
cluster weights on the host (rows are unit-normed, so the global norm is
||cw||_2 exactly). Per sample:
  scoreT[n,K] = qh^T xh + ql^T xh   (fp16 matmuls, fp32 PSUM accum)
  prob = softmax over K (fp16)
  descT[K,d] = prob^T @ xT          (fp16 matmuls, fp32 PSUM accum)
  epilogue in [K,D] layout: denom-normalize, subtract centersT,
  intra-L2, weighted by cw/||cw|| -> out[K,D] (host transposes back)
"""
import numpy as np

B, D, N, K = 32, 512, 4096, 64
NCORES = 8
SPC = B // NCORES          # samples per core
DCH = D // 128             # 4 d-chunks
NCH = N // 128             # 32 n-chunks
BPB = 8                    # score chunks per PSUM bank
NB = NCH // BPB            # 4 score banks per sample
NQ = 4                     # DMA quarters per sample (bank granularity)
NQN = N // NQ              # 1024 n per quarter

_COMPILED = {}


def _build():
    import concourse.bass as bass
    import concourse.bacc as bacc
    import concourse.tile as tile
    import concourse.mybir as mybir

    f32 = mybir.dt.float32
    f16 = mybir.dt.float16
    AF = mybir.ActivationFunctionType
    OP = mybir.AluOpType
    AX = mybir.AxisListType

    nc = bacc.Bacc("TRN2", target_bir_lowering=False, debug=False)
    xh_dram = nc.dram_tensor("xh", [SPC, D, N], f16, kind="ExternalInput")
    xT_dram = nc.dram_tensor("xT", [SPC, N, D], f16, kind="ExternalInput")
    qh_dram = nc.dram_tensor("qh", [D, K], f16, kind="ExternalInput")
    ql_dram = nc.dram_tensor("ql", [D, K], f16, kind="ExternalInput")
    cT_dram = nc.dram_tensor("cT", [K, D], f32, kind="ExternalInput")
    cw_dram = nc.dram_tensor("cw", [K, 1], f32, kind="ExternalInput")
    out_dram = nc.dram_tensor("out", [SPC, K, D], f32, kind="ExternalOutput")

    with tile.TileContext(nc) as tc:
        with (
            tc.tile_pool(name="const", bufs=1) as const,
            tc.tile_pool(name="xhp", bufs=2) as xhp,
            tc.tile_pool(name="xTp", bufs=2) as xTp,
            tc.tile_pool(name="probp", bufs=2) as probp,
            tc.tile_pool(name="smp", bufs=6) as smp,
            tc.tile_pool(name="epp", bufs=1) as epp,
            tc.tile_pool(name="ps_sc", bufs=3, space="PSUM") as ps_sc,
            tc.tile_pool(name="ps_d", bufs=2, space="PSUM") as ps_d,
            tc.tile_pool(name="ps_n", bufs=2, space="PSUM") as ps_n,
        ):
            # ---------- per-sample DMA (quarters, bank granularity) ----
            def load_sample(s):
                xh = xhp.tile([128, DCH, N], f16, tag="xh", name=f"xh{s}")
                xT = xTp.tile([128, NCH, D], f16, tag="xT", name=f"xT{s}")
                return xh, xT

            def load_q(s, q, xh, xT):
                nc.sync.dma_start(
                    xh[:, :, q * NQN:(q + 1) * NQN],
                    xh_dram[s, :, q * NQN:(q + 1) * NQN]
                    .rearrange("(c p) n -> p c n", p=128))
                nc.sync.dma_start(
                    xT[:, q * BPB:(q + 1) * BPB, :],
                    xT_dram[s, q * NQN:(q + 1) * NQN, :]
                    .rearrange("(j p) d -> p j d", p=128))

            # kick off sample 0's x stream before anything else queues
            xcur = load_sample(0)
            load_q(0, 0, *xcur)

            # ---------- one-time prep (tiny; after first x quarter) ----
            qh_sb = const.tile([128, DCH, K], f16, tag="qh_sb")
            nc.sync.dma_start(
                qh_sb[:], qh_dram[:].rearrange("(c p) k -> p c k", p=128))
            ql_sb = const.tile([128, DCH, K], f16, tag="ql_sb")
            nc.sync.dma_start(
                ql_sb[:], ql_dram[:].rearrange("(c p) k -> p c k", p=128))
            ones16 = const.tile([128, 1], f16, tag="ones16")
            nc.gpsimd.memset(ones16[:], 1.0)

            for q in range(1, NQ):
                load_q(0, q, *xcur)
            cT_sb = const.tile([K, D], f32, tag="cT_sb")
            nc.gpsimd.dma_start(cT_sb[:], cT_dram[:])
            cw_sb = const.tile([K, 1], f32, tag="cw_sb")
            nc.gpsimd.dma_start(cw_sb[:], cw_dram[:])

            pending = []  # deferred epilogues

            for s in range(SPC):
                xh, xT = xcur
                descT = ps_d.tile([K, D], f32, tag="descT", name=f"dT{s}")
                denom = ps_n.tile([K, 1], f32, tag="denom", name=f"dn{s}")
                probs = probp.tile([128, NCH, K], f16, tag="prob",
                                   name=f"pr{s}")

                def mm1_bank(b):
                    bank = ps_sc.tile([128, BPB, K], f32, tag="scoreT",
                                      name=f"scb_{s}_{b}")
                    first = [True]

                    def mm(c, lhsT, rhs, last=False):
                        nc.tensor.matmul(
                            bank[:, c, :], lhsT, rhs,
                            start=first[0], stop=last,
                            skip_group_check=(not first[0]))
                        first[0] = False

                    for dc in range(DCH):
                        for c in range(BPB):
                            j = b * BPB + c
                            sl = slice(j * 128, (j + 1) * 128)
                            last = (dc == DCH - 1 and c == BPB - 1)
                            mm(c, xh[:, dc, sl], qh_sb[:, dc, :])
                            mm(c, xh[:, dc, sl], ql_sb[:, dc, :], last=last)
                    return bank

                def softmax_bank(b, bank):
                    negmax = smp.tile([128, BPB], f32, tag="negmax")
                    nc.vector.reduce_max(negmax[:].unsqueeze(2),
                                         bank[:], axis=AX.X, negate=True)
                    zc = smp.tile([128, BPB, K], f32, tag="zc")
                    nc.vector.tensor_add(
                        zc[:], bank[:],
                        negmax[:].unsqueeze(2).broadcast_to([128, BPB, K]))
                    e16 = smp.tile([128, BPB, K], f16, tag="e16")
                    nc.scalar.activation(e16[:].rearrange("p a b -> p (a b)"),
                                         zc[:].rearrange("p a b -> p (a b)"),
                                         AF.Exp)
                    rs = smp.tile([128, BPB], f32, tag="rs")
                    nc.vector.reduce_sum(rs[:].unsqueeze(2), e16[:], axis=AX.X)
                    rr = smp.tile([128, BPB], f32, tag="rr")
                    nc.vector.reciprocal(rr[:], rs[:])
                    nc.gpsimd.tensor_mul(
                        probs[:, b * BPB:(b + 1) * BPB, :], e16[:],
                        rr[:].unsqueeze(2).broadcast_to([128, BPB, K]))

                def mm2_bank(b):
                    for c in range(BPB):
                        j = b * BPB + c
                        nc.tensor.matmul(descT[:], probs[:, j, :],
                                         xT[:, j, :],
                                         start=(j == 0), stop=(j == NCH - 1))
                    for c in range(BPB):
                        j = b * BPB + c
                        nc.tensor.matmul(denom[:], probs[:, j, :], ones16[:],
                                         start=(j == 0), stop=(j == NCH - 1))

                xnext = None
                for g in range(NB // 2):
                    b0, b1 = 2 * g, 2 * g + 1
                    bank0 = mm1_bank(b0)
                    bank1 = mm1_bank(b1)
                    if g == 1:
                        mm2_bank(0)
                        mm2_bank(1)
                    softmax_bank(b0, bank0)
                    softmax_bank(b1, bank1)
                    if g == 1 and pending:
                        pending.pop(0)()
                    # quarter-wise prefetch of the next sample
                    if s + 1 < SPC:
                        if g == 0:
                            xnext = load_sample(s + 1)
                        load_q(s + 1, b0, *xnext)
                        load_q(s + 1, b1, *xnext)
                mm2_bank(NB - 2)
                mm2_bank(NB - 1)

                if s + 1 < SPC:
                    xcur = xnext

                def make_epilogue(s, descT, denom):
                    def run():
                        # ------- epilogue (descT [K, D] layout) -------
                        rdenom = epp.tile([K, 1], f32, tag="rdenom",
                                          name=f"rd{s}")
                        nc.vector.tensor_scalar_max(rdenom[:], denom[:], 1e-6)
                        nc.vector.reciprocal(rdenom[:], rdenom[:])
                        desc_c = epp.tile([K, D], f32, tag="desc_c",
                                          name=f"dcc{s}")
                        nc.vector.scalar_tensor_tensor(
                            desc_c[:], in0=descT[:], scalar=rdenom[:],
                            in1=cT_sb[:], op0=OP.mult, op1=OP.subtract)
                        sqe = epp.tile([K, D], f32, tag="sqe", name=f"sq{s}")
                        nc.gpsimd.tensor_mul(sqe[:], desc_c[:], desc_c[:])
                        ss = epp.tile([K, 1], f32, tag="ss", name=f"ss{s}")
                        nc.vector.reduce_sum(ss[:], sqe[:], axis=AX.X)
                        intra = epp.tile([K, 1], f32, tag="intra",
                                         name=f"in{s}")
                        nc.scalar.activation(intra[:], ss[:], AF.Sqrt)
                        nc.vector.tensor_scalar_max(intra[:], intra[:], 1e-12)
                        rintra = epp.tile([K, 1], f32, tag="rintra",
                                          name=f"ri{s}")
                        nc.vector.reciprocal(rintra[:], intra[:])
                        sfin = epp.tile([K, 1], f32, tag="sfin",
                                        name=f"sf{s}")
                        nc.vector.tensor_mul(sfin[:], cw_sb[:], rintra[:])
                        outT = epp.tile([K, D], f32, tag="outT",
                                        name=f"oT{s}")
                        nc.gpsimd.tensor_mul(outT[:], desc_c[:],
                                             sfin[:].broadcast_to([K, D]))
                        nc.gpsimd.dma_start(out_dram[s], outT[:])
                    return run

                pending.append(make_epilogue(s, descT, denom))
                if s == SPC - 1:
                    for fn in pending:
                        fn()
                    pending.clear()

    nc.compile()
    return nc


def kernel(x, centers, alpha, cluster_weights):
    import concourse.bass_utils as bass_utils

    if "nc" not in _COMPILED:
        _COMPILED["nc"] = _build()
    nc = _COMPILED["nc"]

    x = np.asarray(x, dtype=np.float32)
    xh = np.ascontiguousarray(x.astype(np.float16))
    xT = np.ascontiguousarray(xh.transpose(0, 2, 1))

    c = np.asarray(centers, dtype=np.float64).reshape(D, K)
    a = float(np.asarray(alpha, dtype=np.float64))
    nrm = np.sqrt((c * c).sum(axis=0, keepdims=True))
    q = a * c / np.maximum(nrm, 1e-12)
    qh = q.astype(np.float16)
    ql = (q - qh.astype(np.float64)).astype(np.float16)
    cT = np.ascontiguousarray(c.T.astype(np.float32))
    cw = np.asarray(cluster_weights, dtype=np.float64).reshape(K, 1)
    # rows of desc are unit-L2 then scaled by cw, so the flattened norm
    # is ||cw||_2 exactly: fold the final normalize into cw.
    cw_eff = (cw / max(np.sqrt((cw * cw).sum()), 1e-12)).astype(np.float32)

    in_maps = []
    for core in range(NCORES):
        in_maps.append({
            "xh": xh[core * SPC:(core + 1) * SPC],
            "xT": xT[core * SPC:(core + 1) * SPC],
            "qh": qh,
            "ql": ql,
            "cT": cT,
            "cw": cw_eff,
        })
    res = bass_utils.run_bass_kernel_spmd(nc, in_maps,
                                          core_ids=list(range(NCORES)))
    out = np.concatenate([res.results[i]["out"] for i in range(NCORES)],
                         axis=0)                       # [B, K, D]
    return np.ascontiguousarray(
        out.transpose(0, 2, 1).reshape(B, D * K)).astype(np.float32)


# revision 11
# speedup vs baseline: 1.3149x; 1.3149x over previous
"""AttnVLAD layer on 8 Trainium2 NeuronCores.

Data-parallel over batch: b=32 samples -> 4 per core. Host precomputes
fp16 copies of x in both layouts (d-major for mm1, n-major for mm2) plus
the fp16 split of q = alpha * centers/||centers||, so the device does no
casting or transposing of x. The global L2 normalize is folded into the
cluster weights on the host (rows are unit-normed, so the global norm is
||cw||_2 exactly). Per sample:
  scoreT[n,K] = qh^T xh             (fp16 matmuls, fp32 PSUM accum)
  prob = softmax over K (fp16)
  descT[K,d] = prob^T @ xT          (fp16 matmuls, fp32 PSUM accum)
  epilogue in [K,D] layout: denom-normalize, subtract centersT,
  intra-L2, weighted by cw/||cw|| -> out[K,D] (host transposes back)
"""
import numpy as np

B, D, N, K = 32, 512, 4096, 64
NCORES = 8
SPC = B // NCORES          # samples per core
DCH = D // 128             # 4 d-chunks
NCH = N // 128             # 32 n-chunks
BPB = 8                    # score chunks per PSUM bank
NB = NCH // BPB            # 4 score banks per sample
NQ = 4                     # DMA quarters per sample (bank granularity)
NQN = N // NQ              # 1024 n per quarter

_COMPILED = {}


def _build():
    import concourse.bass as bass
    import concourse.bacc as bacc
    import concourse.tile as tile
    import concourse.mybir as mybir

    f32 = mybir.dt.float32
    f16 = mybir.dt.float16
    AF = mybir.ActivationFunctionType
    OP = mybir.AluOpType
    AX = mybir.AxisListType

    nc = bacc.Bacc("TRN2", target_bir_lowering=False, debug=False)
    xh_dram = nc.dram_tensor("xh", [SPC, D, N], f16, kind="ExternalInput")
    xT_dram = nc.dram_tensor("xT", [SPC, N, D], f16, kind="ExternalInput")
    qh_dram = nc.dram_tensor("qh", [D, K], f16, kind="ExternalInput")
    cT_dram = nc.dram_tensor("cT", [K, D], f32, kind="ExternalInput")
    cw_dram = nc.dram_tensor("cw", [K, 1], f32, kind="ExternalInput")
    out_dram = nc.dram_tensor("out", [SPC, K, D], f32, kind="ExternalOutput")

    with tile.TileContext(nc) as tc:
        with (
            tc.tile_pool(name="const", bufs=1) as const,
            tc.tile_pool(name="xhp", bufs=2) as xhp,
            tc.tile_pool(name="xTp", bufs=2) as xTp,
            tc.tile_pool(name="probp", bufs=2) as probp,
            tc.tile_pool(name="smp", bufs=6) as smp,
            tc.tile_pool(name="epp", bufs=1) as epp,
            tc.tile_pool(name="ps_sc", bufs=3, space="PSUM") as ps_sc,
            tc.tile_pool(name="ps_d", bufs=2, space="PSUM") as ps_d,
            tc.tile_pool(name="ps_n", bufs=2, space="PSUM") as ps_n,
        ):
            # ---------- per-sample DMA (quarters, bank granularity) ----
            def load_sample(s):
                xh = xhp.tile([128, DCH, N], f16, tag="xh", name=f"xh{s}")
                xT = xTp.tile([128, NCH, D], f16, tag="xT", name=f"xT{s}")
                return xh, xT

            def load_q(s, q, xh, xT):
                nc.sync.dma_start(
                    xh[:, :, q * NQN:(q + 1) * NQN],
                    xh_dram[s, :, q * NQN:(q + 1) * NQN]
                    .rearrange("(c p) n -> p c n", p=128))
                nc.sync.dma_start(
                    xT[:, q * BPB:(q + 1) * BPB, :],
                    xT_dram[s, q * NQN:(q + 1) * NQN, :]
                    .rearrange("(j p) d -> p j d", p=128))

            # kick off sample 0's x stream before anything else queues
            xcur = load_sample(0)
            load_q(0, 0, *xcur)

            # ---------- one-time prep (tiny; after first x quarter) ----
            qhl_sb = const.tile([128, DCH, K], f16, tag="qhl_sb")
            nc.sync.dma_start(
                qhl_sb[:], qh_dram[:].rearrange("(c p) k -> p c k", p=128))
            ones16 = const.tile([128, 1], f16, tag="ones16")
            nc.gpsimd.memset(ones16[:], 1.0)

            for q in range(1, NQ):
                load_q(0, q, *xcur)
            cT_sb = const.tile([K, D], f32, tag="cT_sb")
            nc.sync.dma_start(cT_sb[:], cT_dram[:])
            cw_sb = const.tile([K, 1], f32, tag="cw_sb")
            nc.sync.dma_start(cw_sb[:], cw_dram[:])

            pending = []  # deferred epilogues

            for s in range(SPC):
                xh, xT = xcur
                descT = ps_d.tile([K, D], f32, tag="descT", name=f"dT{s}")
                denom = ps_n.tile([K, 1], f32, tag="denom", name=f"dn{s}")
                probs = probp.tile([128, NCH, K], f16, tag="prob",
                                   name=f"pr{s}")

                def mm1_bank(b):
                    bank = ps_sc.tile([128, BPB, K], f32, tag="scoreT",
                                      name=f"scb_{s}_{b}")
                    first = [True]

                    def mm(c, lhsT, rhs, last=False):
                        nc.tensor.matmul(
                            bank[:, c, :], lhsT, rhs,
                            start=first[0], stop=last,
                            skip_group_check=(not first[0]))
                        first[0] = False

                    for dc in range(DCH):
                        for c in range(BPB):
                            j = b * BPB + c
                            sl = slice(j * 128, (j + 1) * 128)
                            last = (dc == DCH - 1 and c == BPB - 1)
                            mm(c, xh[:, dc, sl], qhl_sb[:, dc, :], last=last)
                    return bank

                def softmax_bank(b, bank):
                    negmax = smp.tile([128, BPB], f32, tag="negmax")
                    nc.vector.reduce_max(negmax[:].unsqueeze(2),
                                         bank[:], axis=AX.X, negate=True)
                    e16 = smp.tile([128, BPB, K], f16, tag="e16")
                    for c in range(BPB):
                        nc.scalar.activation(e16[:, c, :], bank[:, c, :],
                                             AF.Exp,
                                             bias=negmax[:, c:c + 1])
                    rs = smp.tile([128, BPB], f32, tag="rs")
                    nc.vector.reduce_sum(rs[:].unsqueeze(2), e16[:], axis=AX.X)
                    rr = smp.tile([128, BPB], f32, tag="rr")
                    nc.vector.reciprocal(rr[:], rs[:])
                    nc.vector.tensor_mul(
                        probs[:, b * BPB:(b + 1) * BPB, :], e16[:],
                        rr[:].unsqueeze(2).broadcast_to([128, BPB, K]))

                def mm2_bank(b):
                    for c in range(BPB):
                        j = b * BPB + c
                        nc.tensor.matmul(descT[:], probs[:, j, :],
                                         xT[:, j, :],
                                         start=(j == 0), stop=(j == NCH - 1))
                    for c in range(BPB):
                        j = b * BPB + c
                        nc.tensor.matmul(denom[:], probs[:, j, :], ones16[:],
                                         start=(j == 0), stop=(j == NCH - 1))

                xnext = None
                for b in range(NB):
                    bank = mm1_bank(b)
                    softmax_bank(b, bank)
                    if b >= 2:
                        mm2_bank(b - 2)
                    if b == 2 and pending:
                        pending.pop(0)()
                    # quarter-wise prefetch of the next sample
                    if s + 1 < SPC:
                        if b == 0:
                            xnext = load_sample(s + 1)
                        load_q(s + 1, b, *xnext)
                mm2_bank(NB - 2)
                mm2_bank(NB - 1)

                if s + 1 < SPC:
                    xcur = xnext

                def make_epilogue(s, descT, denom):
                    def run():
                        # ------- epilogue (descT [K, D] layout) -------
                        rdenom = epp.tile([K, 1], f32, tag="rdenom",
                                          name=f"rd{s}")
                        nc.vector.tensor_scalar_max(rdenom[:], denom[:], 1e-6)
                        nc.vector.reciprocal(rdenom[:], rdenom[:])
                        desc_c = epp.tile([K, D], f32, tag="desc_c",
                                          name=f"dcc{s}")
                        nc.vector.scalar_tensor_tensor(
                            desc_c[:], in0=descT[:], scalar=rdenom[:],
                            in1=cT_sb[:], op0=OP.mult, op1=OP.subtract)
                        sqe = epp.tile([K, D], f32, tag="sqe", name=f"sq{s}")
                        nc.vector.tensor_mul(sqe[:], desc_c[:], desc_c[:])
                        ss = epp.tile([K, 1], f32, tag="ss", name=f"ss{s}")
                        nc.vector.reduce_sum(ss[:], sqe[:], axis=AX.X)
                        intra = epp.tile([K, 1], f32, tag="intra",
                                         name=f"in{s}")
                        nc.scalar.activation(intra[:], ss[:], AF.Sqrt)
                        nc.vector.tensor_scalar_max(intra[:], intra[:], 1e-12)
                        rintra = epp.tile([K, 1], f32, tag="rintra",
                                          name=f"ri{s}")
                        nc.vector.reciprocal(rintra[:], intra[:])
                        sfin = epp.tile([K, 1], f32, tag="sfin",
                                        name=f"sf{s}")
                        nc.vector.tensor_mul(sfin[:], cw_sb[:], rintra[:])
                        outT = epp.tile([K, D], f32, tag="outT",
                                        name=f"oT{s}")
                        nc.vector.tensor_mul(outT[:], desc_c[:],
                                             sfin[:].broadcast_to([K, D]))
                        nc.sync.dma_start(out_dram[s], outT[:])
                    return run

                pending.append(make_epilogue(s, descT, denom))
                if s == SPC - 1:
                    for fn in pending:
                        fn()
                    pending.clear()

    nc.compile()
    return nc


def kernel(x, centers, alpha, cluster_weights):
    import concourse.bass_utils as bass_utils

    if "nc" not in _COMPILED:
        _COMPILED["nc"] = _build()
    nc = _COMPILED["nc"]

    x = np.asarray(x, dtype=np.float32)
    xh = np.ascontiguousarray(x.astype(np.float16))
    xT = np.ascontiguousarray(xh.transpose(0, 2, 1))

    c = np.asarray(centers, dtype=np.float64).reshape(D, K)
    a = float(np.asarray(alpha, dtype=np.float64))
    nrm = np.sqrt((c * c).sum(axis=0, keepdims=True))
    q = a * c / np.maximum(nrm, 1e-12)
    qh = q.astype(np.float16)
    cT = np.ascontiguousarray(c.T.astype(np.float32))
    cw = np.asarray(cluster_weights, dtype=np.float64).reshape(K, 1)
    # rows of desc are unit-L2 then scaled by cw, so the flattened norm
    # is ||cw||_2 exactly: fold the final normalize into cw.
    cw_eff = (cw / max(np.sqrt((cw * cw).sum()), 1e-12)).astype(np.float32)

    in_maps = []
    for core in range(NCORES):
        in_maps.append({
            "xh": xh[core * SPC:(core + 1) * SPC],
            "xT": xT[core * SPC:(core + 1) * SPC],
            "qh": qh,
            "cT": cT,
            "cw": cw_eff,
        })
    res = bass_utils.run_bass_kernel_spmd(nc, in_maps,
                                          core_ids=list(range(NCORES)))
    out = np.concatenate([res.results[i]["out"] for i in range(NCORES)],
                         axis=0)                       # [B, K, D]
    return np.ascontiguousarray(
        out.transpose(0, 2, 1).reshape(B, D * K)).astype(np.float32)
